# revision 35
# baseline (speedup 1.0000x reference)
"""Trainium2 Bass kernel for nn_Dimer2D: log(lambda_max(Wang)/lambda_max(Gong)).

Structure: with As = 0.5*(A + A^T) (symmetric 64x64 A0, A1), matvecs of the
dense Wang (8192^2) / Gong (4096^2) operators factor into a few 64-wide
matmuls.  The device runs short block-Krylov chains; the host does a full
Galerkin Rayleigh-Ritz (fp64) over the pooled raw bases (plus the one
Krylov extension already implied by the matvec products the projection
needs), so device vectors only need to SPAN the right Krylov subspaces.

Device recurrence (norm-scaled two-term chain, bf16 state, fp32 PSUM):
    n_i   = ||u_i||^2       (DVE square-accum + PE ones-reduce + reciprocal)
    u_+1  = (M u_i)/n_i - u_{i-1}
The Krylov span is recurrence-coefficient independent; the adaptive 1/n
scaling alone keeps the bf16 basis well enough conditioned (verified:
adding the Lanczos alpha-orthogonalization term changes the final error
by <1% at these depths).

Matvec layout: state S [128,64], slot matrices stacked on partitions.
  st1:    praw[0:64]  = S^T c1   (= X),  praw[64:128] = S^T c2  (= Y)
  stage2: w = big2^T [X;Y]  (ONE matmul, stationary 128x128 block matrix)
The stage2 output is the matvec in the *other* storage convention
(transposed <-> plain slots alternate each step), and the same constants
serve both parities, so no per-parity code is needed on device; the host
un-transposes the even-step vectors.  Per-core constants select Wang
(c1=[A1;A0] c2=[A0;0] big2=[[A0,0],[A1,A0]]) or Gong (c1=[A0;0]
c2=[A1;0] big2=[[A0,0],[A1,0]], state [V^T;0] / [W;0]).

Cores 0-4 run Wang chains, cores 5-7 Gong; N_CHAINS=2 chains with
different random starts interleave per core (engines are latency-bound,
the second chain rides in the idle slots).  Pooled-basis Galerkin =
block Lanczos on the union Krylov subspace (block width 10 Wang / 6
Gong), which converges in far fewer sequential steps than one chain;
K_DEV=9 device steps + the host extension reach ~7e-4 output error
(tolerance 2e-2), HW-verified to match the numpy bf16 simulation to
~1e-5.

Critical path per chain step: st1(2 matmul) -> Pcopy(DVE tensor_copy;
ACT pays 222-cycle SBUF access vs DVE's 58) -> stage2(1 matmul) ->
u_next(DVE stt), ~1.13us; the norm chain hides under it.  All inputs
arrive in one SP-queue DMA (the ACT queue's first DMA would queue
behind a ~1.3us activation-table load); `ones` is memset; stream DMAs
alternate the SP and gpsimd DGE queues so no single queue's fixed
per-DMA chain (~2.2us) backlogs across the ~1.1us periods.
"""

import numpy as np

K_DEV = 9         # streamed basis vectors per chain (K_DEV-1 compute periods)
N_CHAINS = 2      # chains per core
D = 64
P2 = 128
N_WANG = 5        # cores 0..N_WANG-1 -> Wang, rest -> Gong
WARM = (0, 0)     # PE warm-up dummy matmuls per period (gap sites a, b)
PCOPY_DVE = True   # PSUM->SBUF stage-copy engine: DVE vs ACT
SQ_ACT = False     # norm square-accum engine: ACT vs DVE

_PROGRAM_CACHE = {}


def build_program(k_dev=None, n_chains=None, warm=None):
    k_dev = K_DEV if k_dev is None else k_dev
    n_chains = N_CHAINS if n_chains is None else n_chains
    WARM_A, WARM_B = (WARM if warm is None else warm)
    key = (k_dev, n_chains, WARM_A, WARM_B)
    if key in _PROGRAM_CACHE:
        return _PROGRAM_CACHE[key]

    from contextlib import ExitStack

    import concourse.bacc as bacc
    import concourse.mybir as mybir
    import concourse.tile as tile

    f32 = mybir.dt.float32
    bf16 = mybir.dt.bfloat16
    Alu = mybir.AluOpType
    ActFn = mybir.ActivationFunctionType

    nc = bacc.Bacc("TRN2", target_bir_lowering=False, debug=False, num_devices=8)

    K, NC = k_dev, n_chains
    # cin: [v0_0 | .. | v0_{NC-1} | st1c(128) | big2(128)]
    CIN = NC * D + 4 * D
    cin_d = nc.dram_tensor("cin", [P2, CIN], bf16, kind="ExternalInput").ap()
    vs_d = nc.dram_tensor("vs", [K, P2, NC * D], bf16, kind="ExternalOutput").ap()

    with tile.TileContext(nc) as tc, ExitStack() as ctx:
        cpool = ctx.enter_context(tc.tile_pool(name="consts", bufs=1))
        u_pool = ctx.enter_context(tc.tile_pool(name="u", bufs=4))
        work = ctx.enter_context(tc.tile_pool(name="work", bufs=2 * NC))
        ps_raw = ctx.enter_context(
            tc.tile_pool(name="ps_raw", bufs=1, space="PSUM"))
        ps_w = ctx.enter_context(tc.tile_pool(name="ps_w", bufs=1, space="PSUM"))
        ps_n = ctx.enter_context(tc.tile_pool(name="ps_n", bufs=1, space="PSUM"))
        ps_d = ctx.enter_context(tc.tile_pool(name="ps_d", bufs=1, space="PSUM"))

        cin = cpool.tile([P2, CIN], bf16, name="cin")
        st1c = cin[:, NC * D:NC * D + 2 * D]
        big2 = cin[:, NC * D + 2 * D:NC * D + 4 * D]
        ones = cpool.tile([P2, P2], f32, name="ones")
        zeros = cpool.tile([P2, D], bf16, name="zeros")

        nc.sync.dma_start(cin[:], cin_d)
        nc.vector.memset(ones[:], 1.0)
        nc.vector.memset(zeros[:], 0.0)

        ucur_full = cin[:, 0:NC * D]
        uslice = (lambda t: (lambda c: t[:, c * D:(c + 1) * D]))(cin)
        uprevs = [zeros for _ in range(NC)]

        for i in range(K - 1):
            # alternate stream DMAs across the SP/ACT DGE queues: each queue
            # chain (DGE delay + transfer + sem prop) is ~1.65us, longer than
            # one period, so a single queue would accumulate backlog that the
            # final stream pays for.  Period 0 stays on SP (the ACT queue's
            # first DMA would sit behind the activation-table load).
            eng = nc.sync if i % 2 == 0 else nc.gpsimd
            eng.dma_start(vs_d[i], ucur_full)
            unext = u_pool.tile([P2, NC * D], bf16, tag="u", name=f"u_{i + 1}")

            # DVE square-accum first (DVE queue: sq_c ... recip_c ... u_next_c)
            psqs = []
            for c in range(NC):
                u = uslice(c)
                scr1 = work.tile([P2, D], bf16, tag=f"scr1_{c}", name=f"scr1_{c}_{i}")
                psq = work.tile([P2, 1], f32, tag=f"psq_{c}", name=f"psq_{c}_{i}")
                if SQ_ACT:
                    nc.scalar.activation(scr1[:], u, ActFn.Square,
                                         accum_out=psq[:])
                else:
                    nc.vector.scalar_tensor_tensor(
                        scr1[:], u, 1.0, u, op0=Alu.mult, op1=Alu.mult,
                        accum_out=psq[:],
                    )
                psqs.append(psq)

            # early periods: st1 before the norm reduces so the PE wait queue
            # isn't head-blocked while the input DMAs land
            praws = [None] * NC
            if i < 2:
                for c in range(NC):
                    u = uslice(c)
                    praw = ps_raw.tile([P2, D], f32, tag=f"praw_{c}",
                                       name=f"praw_{c}_{i}")
                    nc.tensor.matmul(praw[0:D, :], u, st1c[:, 0:D], start=True,
                                     stop=True)
                    nc.tensor.matmul(praw[D:P2, :], u, st1c[:, D:2 * D],
                                     start=True, stop=True)
                    praws[c] = praw

            invns = []
            for c in range(NC):
                n_ps = ps_n.tile([P2, 1], f32, tag=f"n_{c}", name=f"n_{c}_{i}")
                nc.tensor.matmul(n_ps[:], ones[:], psqs[c][:], start=True,
                                 stop=True)
                invn = work.tile([P2, 1], f32, tag=f"invn_{c}", name=f"invn_{c}_{i}")
                nc.vector.reciprocal(invn[:], n_ps[:])
                invns.append(invn)

            # matvec chains, phase-ordered so the PE queue is
            # st1A,st1B,[warm],st2A,st2B,[warm]
            Ps, wps = [], []
            for c in range(NC):
                if praws[c] is None:
                    praw = ps_raw.tile([P2, D], f32, tag=f"praw_{c}",
                                       name=f"praw_{c}_{i}")
                    nc.tensor.matmul(praw[0:D, :], uslice(c), st1c[:, 0:D],
                                     start=True, stop=True)
                    nc.tensor.matmul(praw[D:P2, :], uslice(c), st1c[:, D:2 * D],
                                     start=True, stop=True)
                    praws[c] = praw
                P = work.tile([P2, D], bf16, tag=f"P_{c}", name=f"P_{c}_{i}")
                if PCOPY_DVE:
                    nc.vector.tensor_copy(P[:], praws[c][:])
                else:
                    nc.scalar.copy(P[:], praws[c][:])
                Ps.append(P)
            for _ in range(WARM_A):
                dmy = ps_d.tile([D, D], f32, tag="dmy", name=f"dmy_{i}_{_}")
                nc.tensor.matmul(dmy[:], ones[:, 0:D], ones[:, 0:D],
                                 start=True, stop=True)
            for c in range(NC):
                w_ps = ps_w.tile([P2, D], f32, tag=f"w_{c}", name=f"w_{c}_{i}")
                nc.tensor.matmul(w_ps[:], big2[:], Ps[c][:], start=True, stop=True)
                wps.append(w_ps)
            for _ in range(WARM_B):
                dmy = ps_d.tile([D, D], f32, tag="dmy", name=f"dmyb_{i}_{_}")
                nc.tensor.matmul(dmy[:], ones[:, 0:D], ones[:, 0:D],
                                 start=True, stop=True)
            for c in range(NC):
                nc.vector.scalar_tensor_tensor(
                    unext[:, c * D:(c + 1) * D], wps[c][:], invns[c][:],
                    uprevs[c], op0=Alu.mult, op1=Alu.subtract,
                )
                uprevs[c] = uslice(c)

            ucur_full = unext[:]
            uslice = (lambda t: (lambda c: t[:, c * D:(c + 1) * D]))(unext)

        eng = nc.sync if (K - 1) % 2 == 0 else nc.gpsimd
        eng.dma_start(vs_d[K - 1], ucur_full)

    nc.compile()
    _PROGRAM_CACHE[key] = nc
    return nc


# ---------------- host side ----------------

def _bf16(x):
    import ml_dtypes
    return np.asarray(x, dtype=np.float32).astype(ml_dtypes.bfloat16)


def _host_prep(A, n_chains=None):
    n_chains = N_CHAINS if n_chains is None else n_chains
    A = np.asarray(A, dtype=np.float32)
    As = (0.5 * (A + np.swapaxes(A, 1, 2))).astype(np.float32)
    A0, A1 = As[0], As[1]
    Z = np.zeros((D, D), np.float32)

    def stack2(top_a, bot_a, top_b, bot_b):
        c1 = np.concatenate([top_a, bot_a], axis=0)
        c2 = np.concatenate([top_b, bot_b], axis=0)
        return np.concatenate([c1, c2], axis=1)

    def v0_state(seed, wang):
        n = 2 * D * D if wang else D * D
        rng = np.random.default_rng(seed)
        v = rng.standard_normal(n).astype(np.float32)
        v /= np.linalg.norm(v)
        if wang:
            V = v.reshape(D, 2 * D)
            return np.concatenate([V[:, 0:D].T, V[:, D:2 * D].T], axis=0)
        V = v.reshape(D, D)
        return np.concatenate([V.T, np.zeros((D, D), np.float32)], axis=0)

    in_maps = []
    for core in range(8):
        wang = core < N_WANG
        base = core * n_chains if wang else (core - N_WANG) * n_chains
        if wang:
            st1c = stack2(A1, A0, A0, Z)
            big2 = stack2(A0, A1, Z, A0)
        else:
            st1c = stack2(A0, Z, A1, Z)
            big2 = stack2(A0, A1, Z, Z)
        v0s = [v0_state(base + c, wang) for c in range(n_chains)]
        in_maps.append({"cin": _bf16(np.concatenate(v0s + [st1c, big2], axis=1))})
    return A0, A1, in_maps


def _wang_mv(A0, A1, vt):
    V0, V1 = vt[..., :, 0:D], vt[..., :, D:2 * D]
    W = np.empty_like(vt)
    W[..., :, 0:D] = A0 @ V1 @ A0 + A0 @ V0 @ A1 + A1 @ V0 @ A0
    W[..., :, D:2 * D] = A0 @ V0 @ A0
    return W


def _gong_mv(A0, A1, vt):
    return A0 @ vt @ A0 + A1 @ vt @ A1


def _galerkin_extended(U, mv_flat):
    """Galerkin RR over span[U, MU] in fp64 (one Krylov extension)."""
    U = U.astype(np.float64)
    W1 = mv_flat(U)
    B = np.concatenate([U, W1], axis=0)
    MB = np.concatenate([W1, mv_flat(W1)], axis=0)
    G = B @ B.T
    H = B @ MB.T
    H = 0.5 * (H + H.T)
    w, Q = np.linalg.eigh(G)
    keep = w > w[-1] * 1e-12
    Q = Q[:, keep] / np.sqrt(w[keep])
    return np.linalg.eigvalsh(Q.T @ H @ Q)[-1]


def _postprocess(A0, A1, vs_list):
    A0d, A1d = A0.astype(np.float64), A1.astype(np.float64)

    def mvW(X):
        return _wang_mv(A0d, A1d, X.reshape(-1, D, 2 * D)).reshape(X.shape)

    def mvG(X):
        return _gong_mv(A0d, A1d, X.reshape(-1, D, D)).reshape(X.shape)

    wang_rows, gong_rows = [], []
    for core, vs in enumerate(vs_list):
        S = np.asarray(vs, dtype=np.float32)      # [K, 128, NC*64]
        K = S.shape[0]
        S = np.stack([S[:, :, c * D:(c + 1) * D]
                      for c in range(S.shape[2] // D)], axis=1)
        # even steps store [V0^T;V1^T], odd steps store [V0;V1]
        par = (np.arange(K) % 2).astype(bool)
        S = np.where(par[:, None, None, None], S,
                     np.swapaxes(S.reshape(K, -1, 2, D, D), 3, 4).reshape(S.shape))
        S = S.reshape(-1, P2, D)
        V0, V1 = S[:, 0:D, :], S[:, D:P2, :]
        if core < N_WANG:
            wang_rows.append(
                np.concatenate([V0, V1], axis=2).reshape(S.shape[0], -1))
        else:
            gong_rows.append(V0.reshape(S.shape[0], -1))
    UW = np.concatenate(wang_rows, axis=0)
    UG = np.concatenate(gong_rows, axis=0)
    lam_w = _galerkin_extended(UW, mvW)
    lam_g = _galerkin_extended(UG, mvG)
    return np.asarray(np.log(np.float32(lam_w) / np.float32(lam_g)),
                      dtype=np.float32)


def run_device(in_maps, trace=False):
    from concourse.bass_utils import run_bass_kernel_spmd

    nc = build_program()
    res = run_bass_kernel_spmd(nc, [dict(m) for m in in_maps], list(range(8)),
                               trace=trace)
    return res


def kernel(A):
    A0, A1, in_maps = _host_prep(A)
    res = run_device(in_maps, trace=False)
    return _postprocess(A0, A1, [res.results[c]["vs"] for c in range(8)])


# revision 37
# speedup vs baseline: 1.0801x; 1.0801x over previous
"""Trainium2 Bass kernel for nn_Dimer2D: log(lambda_max(Wang)/lambda_max(Gong)).

Structure: with As = 0.5*(A + A^T) (symmetric 64x64 A0, A1), matvecs of the
dense Wang (8192^2) / Gong (4096^2) operators factor into a few 64-wide
matmuls.  The device runs short block-Krylov chains; the host does a full
Galerkin Rayleigh-Ritz (fp64) over the pooled raw bases (plus the one
Krylov extension already implied by the matvec products the projection
needs), so device vectors only need to SPAN the right Krylov subspaces.

Device recurrence (norm-scaled two-term chain, bf16 state, fp32 PSUM):
    n_i   = ||u_i||^2       (DVE square-accum + PE ones-reduce + reciprocal)
    u_+1  = (M u_i)/n_i - u_{i-1}
The Krylov span is recurrence-coefficient independent; the adaptive 1/n
scaling alone keeps the bf16 basis well enough conditioned (verified:
adding the Lanczos alpha-orthogonalization term changes the final error
by <1% at these depths).

Matvec layout: state S [128,64], slot matrices stacked on partitions.
  st1:    praw[0:64]  = S^T c1   (= X),  praw[64:128] = S^T c2  (= Y)
  stage2: w = big2^T [X;Y]  (ONE matmul, stationary 128x128 block matrix)
The stage2 output is the matvec in the *other* storage convention
(transposed <-> plain slots alternate each step), and the same constants
serve both parities, so no per-parity code is needed on device; the host
un-transposes the even-step vectors.  Per-core constants select Wang
(c1=[A1;A0] c2=[A0;0] big2=[[A0,0],[A1,A0]]) or Gong (c1=[A0;0]
c2=[A1;0] big2=[[A0,0],[A1,0]], state [V^T;0] / [W;0]).

Cores 0-4 run Wang chains, cores 5-7 Gong; N_CHAINS=2 chains with
different random starts interleave per core (engines are latency-bound,
the second chain rides in the idle slots).  Pooled-basis Galerkin =
block Lanczos on the union Krylov subspace (block width 10 Wang / 6
Gong), which converges in far fewer sequential steps than one chain;
K_DEV=9 device steps + the host extension reach ~7e-4 output error
(tolerance 2e-2), HW-verified to match the numpy bf16 simulation to
~1e-5.

Critical path per chain step: st1(2 matmul) -> Pcopy(DVE tensor_copy;
ACT pays 222-cycle SBUF access vs DVE's 58) -> stage2(1 matmul) ->
u_next(DVE stt), ~1.13us; the norm chain hides under it.  All inputs
arrive in one SP-queue DMA (the ACT queue's first DMA would queue
behind a ~1.3us activation-table load); `ones` is memset; stream DMAs
alternate the SP and gpsimd DGE queues so no single queue's fixed
per-DMA chain (~2.2us) backlogs across the ~1.1us periods.
"""

import numpy as np

K_DEV = 8         # streamed basis vectors per chain (K_DEV-1 compute periods)
N_CHAINS = 2      # chains per core
D = 64
P2 = 128
N_WANG = 5        # cores 0..N_WANG-1 -> Wang, rest -> Gong
WARM = (0, 0)     # PE warm-up dummy matmuls per period (gap sites a, b)
PCOPY_DVE = True   # PSUM->SBUF stage-copy engine: DVE vs ACT
SQ_ACT = False     # norm square-accum engine: ACT vs DVE

_PROGRAM_CACHE = {}


def build_program(k_dev=None, n_chains=None, warm=None):
    k_dev = K_DEV if k_dev is None else k_dev
    n_chains = N_CHAINS if n_chains is None else n_chains
    WARM_A, WARM_B = (WARM if warm is None else warm)
    key = (k_dev, n_chains, WARM_A, WARM_B)
    if key in _PROGRAM_CACHE:
        return _PROGRAM_CACHE[key]

    from contextlib import ExitStack

    import concourse.bacc as bacc
    import concourse.mybir as mybir
    import concourse.tile as tile

    f32 = mybir.dt.float32
    bf16 = mybir.dt.bfloat16
    Alu = mybir.AluOpType
    ActFn = mybir.ActivationFunctionType

    nc = bacc.Bacc("TRN2", target_bir_lowering=False, debug=False, num_devices=8)

    K, NC = k_dev, n_chains
    # cin: [v0_0 | .. | v0_{NC-1} | st1c(128) | big2(128)]
    CIN = NC * D + 4 * D
    cin_d = nc.dram_tensor("cin", [P2, CIN], bf16, kind="ExternalInput").ap()
    vs_d = nc.dram_tensor("vs", [K, P2, NC * D], bf16, kind="ExternalOutput").ap()

    with tile.TileContext(nc) as tc, ExitStack() as ctx:
        cpool = ctx.enter_context(tc.tile_pool(name="consts", bufs=1))
        u_pool = ctx.enter_context(tc.tile_pool(name="u", bufs=4))
        work = ctx.enter_context(tc.tile_pool(name="work", bufs=2 * NC))
        ps_raw = ctx.enter_context(
            tc.tile_pool(name="ps_raw", bufs=1, space="PSUM"))
        ps_w = ctx.enter_context(tc.tile_pool(name="ps_w", bufs=1, space="PSUM"))
        ps_n = ctx.enter_context(tc.tile_pool(name="ps_n", bufs=1, space="PSUM"))
        ps_d = ctx.enter_context(tc.tile_pool(name="ps_d", bufs=1, space="PSUM"))

        cin = cpool.tile([P2, CIN], bf16, name="cin")
        st1c = cin[:, NC * D:NC * D + 2 * D]
        big2 = cin[:, NC * D + 2 * D:NC * D + 4 * D]
        ones = cpool.tile([P2, P2], f32, name="ones")
        zeros = cpool.tile([P2, D], bf16, name="zeros")

        nc.sync.dma_start(cin[:], cin_d)
        nc.vector.memset(ones[:], 1.0)
        nc.vector.memset(zeros[:], 0.0)

        ucur_full = cin[:, 0:NC * D]
        uslice = (lambda t: (lambda c: t[:, c * D:(c + 1) * D]))(cin)
        uprevs = [zeros for _ in range(NC)]

        for i in range(K - 1):
            # alternate stream DMAs across the SP/ACT DGE queues: each queue
            # chain (DGE delay + transfer + sem prop) is ~1.65us, longer than
            # one period, so a single queue would accumulate backlog that the
            # final stream pays for.  Period 0 stays on SP (the ACT queue's
            # first DMA would sit behind the activation-table load).
            eng = nc.sync if i % 2 == 0 else nc.gpsimd
            eng.dma_start(vs_d[i], ucur_full)
            unext = u_pool.tile([P2, NC * D], bf16, tag="u", name=f"u_{i + 1}")

            # DVE square-accum first (DVE queue: sq_c ... recip_c ... u_next_c)
            psqs = []
            for c in range(NC):
                u = uslice(c)
                scr1 = work.tile([P2, D], bf16, tag=f"scr1_{c}", name=f"scr1_{c}_{i}")
                psq = work.tile([P2, 1], f32, tag=f"psq_{c}", name=f"psq_{c}_{i}")
                if SQ_ACT:
                    nc.scalar.activation(scr1[:], u, ActFn.Square,
                                         accum_out=psq[:])
                else:
                    nc.vector.scalar_tensor_tensor(
                        scr1[:], u, 1.0, u, op0=Alu.mult, op1=Alu.mult,
                        accum_out=psq[:],
                    )
                psqs.append(psq)

            # early periods: st1 before the norm reduces so the PE wait queue
            # isn't head-blocked while the input DMAs land
            praws = [None] * NC
            if i < 2:
                for c in range(NC):
                    u = uslice(c)
                    praw = ps_raw.tile([P2, D], f32, tag=f"praw_{c}",
                                       name=f"praw_{c}_{i}")
                    nc.tensor.matmul(praw[0:D, :], u, st1c[:, 0:D], start=True,
                                     stop=True)
                    nc.tensor.matmul(praw[D:P2, :], u, st1c[:, D:2 * D],
                                     start=True, stop=True)
                    praws[c] = praw

            invns = []
            for c in range(NC):
                n_ps = ps_n.tile([P2, 1], f32, tag=f"n_{c}", name=f"n_{c}_{i}")
                nc.tensor.matmul(n_ps[:], ones[:], psqs[c][:], start=True,
                                 stop=True)
                invn = work.tile([P2, 1], f32, tag=f"invn_{c}", name=f"invn_{c}_{i}")
                nc.vector.reciprocal(invn[:], n_ps[:])
                invns.append(invn)

            # matvec chains, phase-ordered so the PE queue is
            # st1A,st1B,[warm],st2A,st2B,[warm]
            Ps, wps = [], []
            for c in range(NC):
                if praws[c] is None:
                    praw = ps_raw.tile([P2, D], f32, tag=f"praw_{c}",
                                       name=f"praw_{c}_{i}")
                    nc.tensor.matmul(praw[0:D, :], uslice(c), st1c[:, 0:D],
                                     start=True, stop=True)
                    nc.tensor.matmul(praw[D:P2, :], uslice(c), st1c[:, D:2 * D],
                                     start=True, stop=True)
                    praws[c] = praw
                P = work.tile([P2, D], bf16, tag=f"P_{c}", name=f"P_{c}_{i}")
                if PCOPY_DVE:
                    nc.vector.tensor_copy(P[:], praws[c][:])
                else:
                    nc.scalar.copy(P[:], praws[c][:])
                Ps.append(P)
            for _ in range(WARM_A):
                dmy = ps_d.tile([D, D], f32, tag="dmy", name=f"dmy_{i}_{_}")
                nc.tensor.matmul(dmy[:], ones[:, 0:D], ones[:, 0:D],
                                 start=True, stop=True)
            for c in range(NC):
                w_ps = ps_w.tile([P2, D], f32, tag=f"w_{c}", name=f"w_{c}_{i}")
                nc.tensor.matmul(w_ps[:], big2[:], Ps[c][:], start=True, stop=True)
                wps.append(w_ps)
            for _ in range(WARM_B):
                dmy = ps_d.tile([D, D], f32, tag="dmy", name=f"dmyb_{i}_{_}")
                nc.tensor.matmul(dmy[:], ones[:, 0:D], ones[:, 0:D],
                                 start=True, stop=True)
            for c in range(NC):
                nc.vector.scalar_tensor_tensor(
                    unext[:, c * D:(c + 1) * D], wps[c][:], invns[c][:],
                    uprevs[c], op0=Alu.mult, op1=Alu.subtract,
                )
                uprevs[c] = uslice(c)

            ucur_full = unext[:]
            uslice = (lambda t: (lambda c: t[:, c * D:(c + 1) * D]))(unext)

        nc.sync.dma_start(vs_d[K - 1], ucur_full)

    nc.compile()
    _PROGRAM_CACHE[key] = nc
    return nc


# ---------------- host side ----------------

def _bf16(x):
    import ml_dtypes
    return np.asarray(x, dtype=np.float32).astype(ml_dtypes.bfloat16)


def _host_prep(A, n_chains=None):
    n_chains = N_CHAINS if n_chains is None else n_chains
    A = np.asarray(A, dtype=np.float32)
    As = (0.5 * (A + np.swapaxes(A, 1, 2))).astype(np.float32)
    A0, A1 = As[0], As[1]
    Z = np.zeros((D, D), np.float32)

    def stack2(top_a, bot_a, top_b, bot_b):
        c1 = np.concatenate([top_a, bot_a], axis=0)
        c2 = np.concatenate([top_b, bot_b], axis=0)
        return np.concatenate([c1, c2], axis=1)

    def v0_state(seed, wang):
        n = 2 * D * D if wang else D * D
        rng = np.random.default_rng(seed)
        v = rng.standard_normal(n).astype(np.float32)
        v /= np.linalg.norm(v)
        if wang:
            V = v.reshape(D, 2 * D)
            return np.concatenate([V[:, 0:D].T, V[:, D:2 * D].T], axis=0)
        V = v.reshape(D, D)
        return np.concatenate([V.T, np.zeros((D, D), np.float32)], axis=0)

    in_maps = []
    for core in range(8):
        wang = core < N_WANG
        base = core * n_chains if wang else (core - N_WANG) * n_chains
        if wang:
            st1c = stack2(A1, A0, A0, Z)
            big2 = stack2(A0, A1, Z, A0)
        else:
            st1c = stack2(A0, Z, A1, Z)
            big2 = stack2(A0, A1, Z, Z)
        v0s = [v0_state(base + c, wang) for c in range(n_chains)]
        in_maps.append({"cin": _bf16(np.concatenate(v0s + [st1c, big2], axis=1))})
    return A0, A1, in_maps


def _wang_mv(A0, A1, vt):
    V0, V1 = vt[..., :, 0:D], vt[..., :, D:2 * D]
    W = np.empty_like(vt)
    W[..., :, 0:D] = A0 @ V1 @ A0 + A0 @ V0 @ A1 + A1 @ V0 @ A0
    W[..., :, D:2 * D] = A0 @ V0 @ A0
    return W


def _gong_mv(A0, A1, vt):
    return A0 @ vt @ A0 + A1 @ vt @ A1


def _galerkin_extended(U, mv_flat):
    """Galerkin RR over span[U, MU] in fp64 (one Krylov extension)."""
    U = U.astype(np.float64)
    W1 = mv_flat(U)
    B = np.concatenate([U, W1], axis=0)
    MB = np.concatenate([W1, mv_flat(W1)], axis=0)
    G = B @ B.T
    H = B @ MB.T
    H = 0.5 * (H + H.T)
    w, Q = np.linalg.eigh(G)
    keep = w > w[-1] * 1e-12
    Q = Q[:, keep] / np.sqrt(w[keep])
    return np.linalg.eigvalsh(Q.T @ H @ Q)[-1]


def _postprocess(A0, A1, vs_list):
    A0d, A1d = A0.astype(np.float64), A1.astype(np.float64)

    def mvW(X):
        return _wang_mv(A0d, A1d, X.reshape(-1, D, 2 * D)).reshape(X.shape)

    def mvG(X):
        return _gong_mv(A0d, A1d, X.reshape(-1, D, D)).reshape(X.shape)

    wang_rows, gong_rows = [], []
    for core, vs in enumerate(vs_list):
        S = np.asarray(vs, dtype=np.float32)      # [K, 128, NC*64]
        K = S.shape[0]
        S = np.stack([S[:, :, c * D:(c + 1) * D]
                      for c in range(S.shape[2] // D)], axis=1)
        # even steps store [V0^T;V1^T], odd steps store [V0;V1]
        par = (np.arange(K) % 2).astype(bool)
        S = np.where(par[:, None, None, None], S,
                     np.swapaxes(S.reshape(K, -1, 2, D, D), 3, 4).reshape(S.shape))
        S = S.reshape(-1, P2, D)
        V0, V1 = S[:, 0:D, :], S[:, D:P2, :]
        if core < N_WANG:
            wang_rows.append(
                np.concatenate([V0, V1], axis=2).reshape(S.shape[0], -1))
        else:
            gong_rows.append(V0.reshape(S.shape[0], -1))
    UW = np.concatenate(wang_rows, axis=0)
    UG = np.concatenate(gong_rows, axis=0)
    lam_w = _galerkin_extended(UW, mvW)
    lam_g = _galerkin_extended(UG, mvG)
    return np.asarray(np.log(np.float32(lam_w) / np.float32(lam_g)),
                      dtype=np.float32)


def run_device(in_maps, trace=False):
    from concourse.bass_utils import run_bass_kernel_spmd

    nc = build_program()
    res = run_bass_kernel_spmd(nc, [dict(m) for m in in_maps], list(range(8)),
                               trace=trace)
    return res


def kernel(A):
    A0, A1, in_maps = _host_prep(A)
    res = run_device(in_maps, trace=False)
    return _postprocess(A0, A1, [res.results[c]["vs"] for c in range(8)])


# revision 38
# speedup vs baseline: 1.1742x; 1.0871x over previous
"""Trainium2 Bass kernel for nn_Dimer2D: log(lambda_max(Wang)/lambda_max(Gong)).

Structure: with As = 0.5*(A + A^T) (symmetric 64x64 A0, A1), matvecs of the
dense Wang (8192^2) / Gong (4096^2) operators factor into a few 64-wide
matmuls.  The device runs short block-Krylov chains; the host does a full
Galerkin Rayleigh-Ritz (fp64) over the pooled raw bases (plus the one
Krylov extension already implied by the matvec products the projection
needs), so device vectors only need to SPAN the right Krylov subspaces.

Device recurrence (norm-scaled two-term chain, bf16 state, fp32 PSUM):
    n_i   = ||u_i||^2       (DVE square-accum + PE ones-reduce + reciprocal)
    u_+1  = (M u_i)/n_i - u_{i-1}
The Krylov span is recurrence-coefficient independent; the adaptive 1/n
scaling alone keeps the bf16 basis well enough conditioned (verified:
adding the Lanczos alpha-orthogonalization term changes the final error
by <1% at these depths).

Matvec layout: state S [128,64], slot matrices stacked on partitions.
  st1:    praw[0:64]  = S^T c1   (= X),  praw[64:128] = S^T c2  (= Y)
  stage2: w = big2^T [X;Y]  (ONE matmul, stationary 128x128 block matrix)
The stage2 output is the matvec in the *other* storage convention
(transposed <-> plain slots alternate each step), and the same constants
serve both parities, so no per-parity code is needed on device; the host
un-transposes the even-step vectors.  Per-core constants select Wang
(c1=[A1;A0] c2=[A0;0] big2=[[A0,0],[A1,A0]]) or Gong (c1=[A0;0]
c2=[A1;0] big2=[[A0,0],[A1,0]], state [V^T;0] / [W;0]).

Cores 0-4 run Wang chains, cores 5-7 Gong; N_CHAINS=2 chains with
different random starts interleave per core (engines are latency-bound,
the second chain rides in the idle slots).  Pooled-basis Galerkin =
block Lanczos on the union Krylov subspace (block width 10 Wang / 6
Gong), which converges in far fewer sequential steps than one chain;
K_DEV=9 device steps + the host extension reach ~7e-4 output error
(tolerance 2e-2), HW-verified to match the numpy bf16 simulation to
~1e-5.

Critical path per chain step: st1(2 matmul) -> Pcopy(DVE tensor_copy;
ACT pays 222-cycle SBUF access vs DVE's 58) -> stage2(1 matmul) ->
u_next(DVE stt), ~1.13us; the norm chain hides under it.  All inputs
arrive in one SP-queue DMA (the ACT queue's first DMA would queue
behind a ~1.3us activation-table load); `ones` is memset; stream DMAs
alternate the SP and gpsimd DGE queues so no single queue's fixed
per-DMA chain (~2.2us) backlogs across the ~1.1us periods.
"""

import numpy as np

K_DEV = 7         # streamed basis vectors per chain (K_DEV-1 compute periods)
N_CHAINS = 2      # chains per core
D = 64
P2 = 128
N_WANG = 5        # cores 0..N_WANG-1 -> Wang, rest -> Gong
WARM = (0, 0)     # PE warm-up dummy matmuls per period (gap sites a, b)
PCOPY_DVE = True   # PSUM->SBUF stage-copy engine: DVE vs ACT
SQ_ACT = False     # norm square-accum engine: ACT vs DVE

_PROGRAM_CACHE = {}


def build_program(k_dev=None, n_chains=None, warm=None):
    k_dev = K_DEV if k_dev is None else k_dev
    n_chains = N_CHAINS if n_chains is None else n_chains
    WARM_A, WARM_B = (WARM if warm is None else warm)
    key = (k_dev, n_chains, WARM_A, WARM_B)
    if key in _PROGRAM_CACHE:
        return _PROGRAM_CACHE[key]

    from contextlib import ExitStack

    import concourse.bacc as bacc
    import concourse.mybir as mybir
    import concourse.tile as tile

    f32 = mybir.dt.float32
    bf16 = mybir.dt.bfloat16
    Alu = mybir.AluOpType
    ActFn = mybir.ActivationFunctionType

    nc = bacc.Bacc("TRN2", target_bir_lowering=False, debug=False, num_devices=8)

    K, NC = k_dev, n_chains
    # cin: [v0_0 | .. | v0_{NC-1} | st1c(128) | big2(128)]
    CIN = NC * D + 4 * D
    cin_d = nc.dram_tensor("cin", [P2, CIN], bf16, kind="ExternalInput").ap()
    vs_d = nc.dram_tensor("vs", [K, P2, NC * D], bf16, kind="ExternalOutput").ap()

    with tile.TileContext(nc) as tc, ExitStack() as ctx:
        cpool = ctx.enter_context(tc.tile_pool(name="consts", bufs=1))
        u_pool = ctx.enter_context(tc.tile_pool(name="u", bufs=4))
        work = ctx.enter_context(tc.tile_pool(name="work", bufs=2 * NC))
        ps_raw = ctx.enter_context(
            tc.tile_pool(name="ps_raw", bufs=1, space="PSUM"))
        ps_w = ctx.enter_context(tc.tile_pool(name="ps_w", bufs=1, space="PSUM"))
        ps_n = ctx.enter_context(tc.tile_pool(name="ps_n", bufs=1, space="PSUM"))
        ps_d = ctx.enter_context(tc.tile_pool(name="ps_d", bufs=1, space="PSUM"))

        cin = cpool.tile([P2, CIN], bf16, name="cin")
        st1c = cin[:, NC * D:NC * D + 2 * D]
        big2 = cin[:, NC * D + 2 * D:NC * D + 4 * D]
        ones = cpool.tile([P2, P2], f32, name="ones")
        zeros = cpool.tile([P2, D], bf16, name="zeros")

        nc.sync.dma_start(cin[:], cin_d)
        nc.vector.memset(ones[:], 1.0)
        nc.vector.memset(zeros[:], 0.0)

        ucur_full = cin[:, 0:NC * D]
        uslice = (lambda t: (lambda c: t[:, c * D:(c + 1) * D]))(cin)
        uprevs = [zeros for _ in range(NC)]

        for i in range(K - 1):
            # alternate stream DMAs across the SP/ACT DGE queues: each queue
            # chain (DGE delay + transfer + sem prop) is ~1.65us, longer than
            # one period, so a single queue would accumulate backlog that the
            # final stream pays for.  Period 0 stays on SP (the ACT queue's
            # first DMA would sit behind the activation-table load).
            eng = nc.sync if i % 2 == 0 else nc.gpsimd
            eng.dma_start(vs_d[i], ucur_full)
            unext = u_pool.tile([P2, NC * D], bf16, tag="u", name=f"u_{i + 1}")

            # DVE square-accum first (DVE queue: sq_c ... recip_c ... u_next_c)
            psqs = []
            for c in range(NC):
                u = uslice(c)
                scr1 = work.tile([P2, D], bf16, tag=f"scr1_{c}", name=f"scr1_{c}_{i}")
                psq = work.tile([P2, 1], f32, tag=f"psq_{c}", name=f"psq_{c}_{i}")
                if SQ_ACT:
                    nc.scalar.activation(scr1[:], u, ActFn.Square,
                                         accum_out=psq[:])
                else:
                    nc.vector.scalar_tensor_tensor(
                        scr1[:], u, 1.0, u, op0=Alu.mult, op1=Alu.mult,
                        accum_out=psq[:],
                    )
                psqs.append(psq)

            # early periods: st1 before the norm reduces so the PE wait queue
            # isn't head-blocked while the input DMAs land
            praws = [None] * NC
            if i < 2:
                for c in range(NC):
                    u = uslice(c)
                    praw = ps_raw.tile([P2, D], f32, tag=f"praw_{c}",
                                       name=f"praw_{c}_{i}")
                    nc.tensor.matmul(praw[0:D, :], u, st1c[:, 0:D], start=True,
                                     stop=True)
                    nc.tensor.matmul(praw[D:P2, :], u, st1c[:, D:2 * D],
                                     start=True, stop=True)
                    praws[c] = praw

            invns = []
            for c in range(NC):
                n_ps = ps_n.tile([P2, 1], f32, tag=f"n_{c}", name=f"n_{c}_{i}")
                nc.tensor.matmul(n_ps[:], ones[:], psqs[c][:], start=True,
                                 stop=True)
                invn = work.tile([P2, 1], f32, tag=f"invn_{c}", name=f"invn_{c}_{i}")
                nc.vector.reciprocal(invn[:], n_ps[:])
                invns.append(invn)

            # matvec chains, phase-ordered so the PE queue is
            # st1A,st1B,[warm],st2A,st2B,[warm]
            Ps, wps = [], []
            for c in range(NC):
                if praws[c] is None:
                    praw = ps_raw.tile([P2, D], f32, tag=f"praw_{c}",
                                       name=f"praw_{c}_{i}")
                    nc.tensor.matmul(praw[0:D, :], uslice(c), st1c[:, 0:D],
                                     start=True, stop=True)
                    nc.tensor.matmul(praw[D:P2, :], uslice(c), st1c[:, D:2 * D],
                                     start=True, stop=True)
                    praws[c] = praw
                P = work.tile([P2, D], bf16, tag=f"P_{c}", name=f"P_{c}_{i}")
                if PCOPY_DVE:
                    nc.vector.tensor_copy(P[:], praws[c][:])
                else:
                    nc.scalar.copy(P[:], praws[c][:])
                Ps.append(P)
            for _ in range(WARM_A):
                dmy = ps_d.tile([D, D], f32, tag="dmy", name=f"dmy_{i}_{_}")
                nc.tensor.matmul(dmy[:], ones[:, 0:D], ones[:, 0:D],
                                 start=True, stop=True)
            for c in range(NC):
                w_ps = ps_w.tile([P2, D], f32, tag=f"w_{c}", name=f"w_{c}_{i}")
                nc.tensor.matmul(w_ps[:], big2[:], Ps[c][:], start=True, stop=True)
                wps.append(w_ps)
            for _ in range(WARM_B):
                dmy = ps_d.tile([D, D], f32, tag="dmy", name=f"dmyb_{i}_{_}")
                nc.tensor.matmul(dmy[:], ones[:, 0:D], ones[:, 0:D],
                                 start=True, stop=True)
            for c in range(NC):
                nc.vector.scalar_tensor_tensor(
                    unext[:, c * D:(c + 1) * D], wps[c][:], invns[c][:],
                    uprevs[c], op0=Alu.mult, op1=Alu.subtract,
                )
                uprevs[c] = uslice(c)

            ucur_full = unext[:]
            uslice = (lambda t: (lambda c: t[:, c * D:(c + 1) * D]))(unext)

        nc.sync.dma_start(vs_d[K - 1], ucur_full)

    nc.compile()
    _PROGRAM_CACHE[key] = nc
    return nc


# ---------------- host side ----------------

def _bf16(x):
    import ml_dtypes
    return np.asarray(x, dtype=np.float32).astype(ml_dtypes.bfloat16)


def _host_prep(A, n_chains=None):
    n_chains = N_CHAINS if n_chains is None else n_chains
    A = np.asarray(A, dtype=np.float32)
    As = (0.5 * (A + np.swapaxes(A, 1, 2))).astype(np.float32)
    A0, A1 = As[0], As[1]
    Z = np.zeros((D, D), np.float32)

    def stack2(top_a, bot_a, top_b, bot_b):
        c1 = np.concatenate([top_a, bot_a], axis=0)
        c2 = np.concatenate([top_b, bot_b], axis=0)
        return np.concatenate([c1, c2], axis=1)

    def v0_state(seed, wang):
        n = 2 * D * D if wang else D * D
        rng = np.random.default_rng(seed)
        v = rng.standard_normal(n).astype(np.float32)
        v /= np.linalg.norm(v)
        if wang:
            V = v.reshape(D, 2 * D)
            return np.concatenate([V[:, 0:D].T, V[:, D:2 * D].T], axis=0)
        V = v.reshape(D, D)
        return np.concatenate([V.T, np.zeros((D, D), np.float32)], axis=0)

    in_maps = []
    for core in range(8):
        wang = core < N_WANG
        base = core * n_chains if wang else (core - N_WANG) * n_chains
        if wang:
            st1c = stack2(A1, A0, A0, Z)
            big2 = stack2(A0, A1, Z, A0)
        else:
            st1c = stack2(A0, Z, A1, Z)
            big2 = stack2(A0, A1, Z, Z)
        v0s = [v0_state(base + c, wang) for c in range(n_chains)]
        in_maps.append({"cin": _bf16(np.concatenate(v0s + [st1c, big2], axis=1))})
    return A0, A1, in_maps


def _wang_mv(A0, A1, vt):
    V0, V1 = vt[..., :, 0:D], vt[..., :, D:2 * D]
    W = np.empty_like(vt)
    W[..., :, 0:D] = A0 @ V1 @ A0 + A0 @ V0 @ A1 + A1 @ V0 @ A0
    W[..., :, D:2 * D] = A0 @ V0 @ A0
    return W


def _gong_mv(A0, A1, vt):
    return A0 @ vt @ A0 + A1 @ vt @ A1


def _galerkin_extended(U, mv_flat):
    """Galerkin RR over span[U, MU] in fp64 (one Krylov extension)."""
    U = U.astype(np.float64)
    W1 = mv_flat(U)
    B = np.concatenate([U, W1], axis=0)
    MB = np.concatenate([W1, mv_flat(W1)], axis=0)
    G = B @ B.T
    H = B @ MB.T
    H = 0.5 * (H + H.T)
    w, Q = np.linalg.eigh(G)
    keep = w > w[-1] * 1e-12
    Q = Q[:, keep] / np.sqrt(w[keep])
    return np.linalg.eigvalsh(Q.T @ H @ Q)[-1]


def _postprocess(A0, A1, vs_list):
    A0d, A1d = A0.astype(np.float64), A1.astype(np.float64)

    def mvW(X):
        return _wang_mv(A0d, A1d, X.reshape(-1, D, 2 * D)).reshape(X.shape)

    def mvG(X):
        return _gong_mv(A0d, A1d, X.reshape(-1, D, D)).reshape(X.shape)

    wang_rows, gong_rows = [], []
    for core, vs in enumerate(vs_list):
        S = np.asarray(vs, dtype=np.float32)      # [K, 128, NC*64]
        K = S.shape[0]
        S = np.stack([S[:, :, c * D:(c + 1) * D]
                      for c in range(S.shape[2] // D)], axis=1)
        # even steps store [V0^T;V1^T], odd steps store [V0;V1]
        par = (np.arange(K) % 2).astype(bool)
        S = np.where(par[:, None, None, None], S,
                     np.swapaxes(S.reshape(K, -1, 2, D, D), 3, 4).reshape(S.shape))
        S = S.reshape(-1, P2, D)
        V0, V1 = S[:, 0:D, :], S[:, D:P2, :]
        if core < N_WANG:
            wang_rows.append(
                np.concatenate([V0, V1], axis=2).reshape(S.shape[0], -1))
        else:
            gong_rows.append(V0.reshape(S.shape[0], -1))
    UW = np.concatenate(wang_rows, axis=0)
    UG = np.concatenate(gong_rows, axis=0)
    lam_w = _galerkin_extended(UW, mvW)
    lam_g = _galerkin_extended(UG, mvG)
    return np.asarray(np.log(np.float32(lam_w) / np.float32(lam_g)),
                      dtype=np.float32)


def run_device(in_maps, trace=False):
    from concourse.bass_utils import run_bass_kernel_spmd

    nc = build_program()
    res = run_bass_kernel_spmd(nc, [dict(m) for m in in_maps], list(range(8)),
                               trace=trace)
    return res


def kernel(A):
    A0, A1, in_maps = _host_prep(A)
    res = run_device(in_maps, trace=False)
    return _postprocess(A0, A1, [res.results[c]["vs"] for c in range(8)])
